# revision 35
# baseline (speedup 1.0000x reference)
"""AdditiveAttention (Bahdanau) Trainium2 Bass kernel.

Math (per batch b):
  qf = queries @ Wq                  (Lq, H)
  kf = keys @ Wk                     (Lk, H)
  scores[q,k] = sum_h wv[h] * tanh(qf[q,h] + kf[k,h])
  attn = softmax(where(mask, -inf, scores), axis=k)
  out  = attn @ values               (Lq, Dv)

Sharding: data-parallel over batch B=8 across the 8 NeuronCores (one
batch per core). All heavy work is fused on-chip; nothing (B,Lq,Lk,H)
sized ever touches HBM.

Per-core plan (Lq=Lk=512, D=256, H=64):
  - PE-transpose queries/keys tiles (f32r) -> qT (d,q), kT (d,k).
  - Xq = [Wq|Wq].T @ qT   -> (128, 512) = qf.T stacked twice (h2 = 2x64).
  - Kst = [Wk|Wk].T @ kT  -> (128, 512); strided-evacuate into
    B2 (128, 256) f32 where column p = [kf[2p,:]; kf[2p+1,:]].
  - Main loop over 256 key-pairs in small sub-batches (8 pairs: keeps
    the PE HAM clock-gate warm; idle gaps stay < the 3.4us window):
      DVE:  A[:, j*512:(j+1)*512] = Xq + B2[:, p]      (bf16)
      ACT:  T = tanh(A)                                 (the roofline)
      PE:   scoresT_psum(128,512) += Wwin_j.T @ T_j     (block-diag wv)
    where Wwin_j is a 128-col window of a (128, 254) buffer holding
    [wv;0] and [0;wv] at columns 126/127 -> the window places wv at
    output partitions 2j, 2j+1.
  - Mask: maskT (k,q) bf16 via PE transpose; folded into scores as the
    final accumulation  scoresT += (-BIG * I).T @ maskT.
  - ACT exp -> E (k, q) f32; PE (f32r): O[qb] += E[:, qb].T @ [values|1]
    giving unnormalized output and the softmax denominator in col 256.
  - DVE: out = O[:, :256] * (1 / O[:, 256]).

kernel(**inputs) takes the FULL unsharded inputs and returns the full
(8, 512, 256) float32 output.
"""

import numpy as np
import ml_dtypes

import concourse.bass as bass
import concourse.mybir as mybir
import concourse.tile as tile
from concourse import bacc
from concourse.bass_utils import run_bass_kernel_spmd
from concourse.masks import make_identity

B, LQ, LK = 8, 512, 512
D, H = 256, 64
DV = 256
NCORES = 8
BIGNEG = 1.0e30           # mask fill magnitude (exp(-BIGNEG) == 0.0 in f32)

F32 = mybir.dt.float32
F32R = mybir.dt.float32r
BF16 = mybir.dt.bfloat16
U8 = mybir.dt.uint8

# tanh sub-batch sizes (in key-pairs) for each of the 4 key blocks.
# 16-pair steady state minimizes the ACT per-instruction overhead while
# keeping PE idle gaps under the 3.4us HAM re-throttle window; the tiny
# first/last batches shorten the pipeline ramp-in and drain.
BATCHES_KB0 = [4, 4, 8, 16, 16, 16]
BATCHES_MID = [16] * 4
BATCHES_KB3 = [16, 16, 16, 8, 4, 4]

_CACHE = {}


def _emit(nc, tc, io):
    from contextlib import ExitStack

    q_d, k_d, vo_d, mask_d = io["q"], io["k"], io["vo"], io["mask"]
    constsf_d, constsb_d = io["constsf"], io["constsb"]
    out_d = io["out"]

    with ExitStack() as ctx:
        ep = ctx.enter_context
        consts = ep(tc.tile_pool(name="consts", bufs=1))
        qkraw = ep(tc.tile_pool(name="qkraw", bufs=1))
        qkT = ep(tc.tile_pool(name="qkT", bufs=2))
        small = ep(tc.tile_pool(name="small", bufs=1))
        abatch = ep(tc.tile_pool(name="abatch", bufs=3))
        tbatch = ep(tc.tile_pool(name="tbatch", bufs=3))
        epool = ep(tc.tile_pool(name="epool", bufs=2))
        mwork = ep(tc.tile_pool(name="mwork", bufs=1))
        mtT = ep(tc.tile_pool(name="mtT", bufs=4))
        votiles = ep(tc.tile_pool(name="votiles", bufs=1))
        outp = ep(tc.tile_pool(name="outp", bufs=2))
        recs = ep(tc.tile_pool(name="recs", bufs=2))
        # PSUM: ps_sc (2 banks) + a prologue-scoped pool (4 banks, closed
        # after mask prep) + ps_o (4 banks) opened for the main loop.
        ps_sc = ep(tc.tile_pool(name="ps_sc", bufs=2, space="PSUM"))
        ps_pre = ep(tc.tile_pool(name="ps_pre", bufs=2, space="PSUM"))
        ps_o = ep(tc.tile_pool(name="ps_o", bufs=4, space="PSUM"))

        # --- constants: identity built on-device; W / mask consts DMA'd ---
        # constsf: [Wq2_c0 | Wq2_c1 | Wk2_c0 | Wk2_c1]  (duplicated cols)
        # constsb: [identity_bf16 | -BIG*identity_bf16 | wv window (254)]
        identf = small.tile([128, 128], F32, tag="identf")
        make_identity(nc, identf[:])
        identr = small.tile([128, 128], F32R, tag="identr")
        nc.vector.tensor_copy(identr[:], identf[:])

        # --- queries/keys: one DMA per 128-row block. 3 parallel DMA
        # rings: GpSimd leads with the W constants + k block 0 (kb0's
        # bias chain), Sync carries q0/q2 + k1/k3, Scalar q1/q3 + k2. ---
        qre = q_d.rearrange("(b p) d -> p b d", b=4)
        kre = k_d.rearrange("(b p) d -> p b d", b=4)
        qraw = qkraw.tile([128, 4, 256], F32, tag="qraw")
        kraw = qkraw.tile([128, 4, 256], F32, tag="kraw")
        cf = consts.tile([128, 512], F32, tag="cf")
        cb = consts.tile([128, 510], BF16, tag="cb")
        nc.gpsimd.dma_start(out=kraw[:, 0, :], in_=kre[:, 0, :])
        nc.sync.dma_start(out=qraw[:, 0, :], in_=qre[:, 0, :])
        nc.scalar.dma_start(out=qraw[:, 1, :], in_=qre[:, 1, :])
        nc.sync.dma_start(out=qraw[:, 2, :], in_=qre[:, 2, :])
        nc.gpsimd.dma_start(out=qraw[:, 3, :], in_=qre[:, 3, :])
        nc.scalar.dma_start(out=cf[:], in_=constsf_d[:])
        nc.sync.dma_start(out=kraw[:, 1, :], in_=kre[:, 1, :])
        nc.scalar.dma_start(out=kraw[:, 2, :], in_=kre[:, 2, :])
        nc.sync.dma_start(out=kraw[:, 3, :], in_=kre[:, 3, :])
        nc.gpsimd.dma_start(out=cb[:], in_=constsb_d[:])
        identb = cb[:, 0:128]
        negib = cb[:, 128:256]
        wvwin = cb[:, 256:510]

        # f32r rounding copies (BIR requires f32r matmul inputs to come
        # from rounding producers). q casts on DVE; k casts on GpSimd so
        # the DVE queue stays clear for the PSUM evacuations.
        wr = small.tile([128, 512], F32R, tag="wr")
        nc.vector.tensor_copy(wr[:], cf[:])
        wq_r = [wr[:, 0:128], wr[:, 128:256]]
        wk_r = [wr[:, 256:384], wr[:, 384:512]]
        qraw_r = qkraw.tile([128, 4, 256], F32R, tag="qraw_r")
        kraw_r = qkraw.tile([128, 4, 256], F32R, tag="kraw_r")
        nc.vector.tensor_copy(kraw_r[:, 0, :], kraw[:, 0, :])
        for blk in range(4):
            nc.vector.tensor_copy(qraw_r[:, blk, :], qraw[:, blk, :])

        # --- transpose q on PE (f32r), fully per-q-block pipelined:
        # each block's transposes, ACT evacuations, Xq matmul pair and
        # xq copy complete as soon as that block's DMA lands (subtile
        # deps), so only the last block's chain sits on the ramp. ---
        qT = [qkT.tile([128, 512], F32R, tag="qkT", name="qT")
              for _ in range(2)]
        # xq_ps borrows a ps_o slot (freed before o_ps[3] is written)
        xq_ps = ps_o.tile([128, 512], F32, tag="o", name="xq_ps")
        bankq = [ps_pre.tile([128, 512], F32R, tag="pre", name="tq")
                 for _ in range(2)]
        xq = small.tile([128, 512], BF16, tag="xq")
        for blk in range(4):
            for db in range(2):
                nc.tensor.transpose(
                    bankq[db][:, blk * 128:(blk + 1) * 128],
                    qraw_r[:, blk, db * 128:(db + 1) * 128],
                    identr[:],
                )
                nc.scalar.copy(
                    qT[db][:, blk * 128:(blk + 1) * 128],
                    bankq[db][:, blk * 128:(blk + 1) * 128],
                )
                nc.tensor.matmul(
                    xq_ps[:, blk * 128:(blk + 1) * 128],
                    wq_r[db], qT[db][:, blk * 128:(blk + 1) * 128],
                    start=(db == 0), stop=(db == 1),
                )
            nc.scalar.copy(
                xq[:, blk * 128:(blk + 1) * 128],
                xq_ps[:, blk * 128:(blk + 1) * 128],
            )

        # --- k chains: transpose + Kst + B2 columns per key-block.
        # Only block 0 is emitted in the prologue (it gates the first
        # tanh); blocks 1-3 are deferred into the kb0 batch region so
        # their late DMAs cannot head-of-line-block the DVE queue. ---
        kst_ps = ps_sc.tile([128, 512], F32, tag="sc", name="kst_ps")
        b2 = small.tile([128, 256], F32, tag="b2")
        kTb = qkT.tile([128, 4, 256], F32R, tag="kTb")

        def emit_k_chain(kbi):
            if kbi > 0:
                nc.vector.tensor_copy(kraw_r[:, kbi, :], kraw[:, kbi, :])
            bank = ps_pre.tile([128, 256], F32R, tag="pre", name="tk")
            for db in range(2):
                nc.tensor.transpose(
                    bank[:, db * 128:(db + 1) * 128],
                    kraw_r[:, kbi, db * 128:(db + 1) * 128],
                    identr[:],
                )
            nc.vector.tensor_copy(kTb[:, kbi, :], bank[:])
            for db in range(2):
                nc.tensor.matmul(
                    kst_ps[:, kbi * 128:(kbi + 1) * 128],
                    wk_r[db], kTb[:, kbi, db * 128:(db + 1) * 128],
                    start=(db == 0), stop=(db == 1),
                )
            nc.vector.tensor_copy(
                b2[0:64, kbi * 64:(kbi + 1) * 64],
                kst_ps[0:64, kbi * 128:(kbi + 1) * 128:2])
            nc.vector.tensor_copy(
                b2[64:128, kbi * 64:(kbi + 1) * 64],
                kst_ps[64:128, kbi * 128 + 1:(kbi + 1) * 128:2])

        emit_k_chain(0)

        # --- values|ones and mask loads (GpSimd SWDGE queue) ---
        vot = votiles.tile([128, 4, DV + 2], F32, tag="vo")
        nc.gpsimd.dma_start(out=vot[:],
                            in_=vo_d.rearrange("(b p) d -> p b d", b=4))
        vot_r = votiles.tile([128, 4, DV + 2], F32R, tag="vor")
        vo = [vot_r[:, kb, :] for kb in range(4)]
        mu8 = mwork.tile([128, 4, 512], U8, tag="mu8")
        nc.gpsimd.dma_start(out=mu8[:],
                            in_=mask_d.rearrange("(b p) d -> p b d", b=4))
        mbf = mwork.tile([128, 4, 512], BF16, tag="mbf")
        maskT = [mtT.tile([128, 512], BF16, tag="mt", name="mt")
                 for _ in range(4)]

        def emit_mask_prep():
            # maskT (k, q) via banked PE transposes; emitted after kb0's
            # tanh batches so it does not steal PE/DVE from the ramp.
            nc.vector.tensor_copy(vot_r[:], vot[:])
            nc.vector.tensor_copy(mbf[:], mu8[:])
            for kb in range(4):
                bank = ps_pre.tile([128, 512], BF16, tag="pre", name="tm")
                for qb in range(4):
                    nc.tensor.transpose(
                        bank[:, qb * 128:(qb + 1) * 128],
                        mbf[:, qb, kb * 128:(kb + 1) * 128],
                        identb,
                    )
                nc.vector.tensor_copy(maskT[kb][:], bank[:])

        # --- main loop: tanh features + blockwise wv reduction ---
        o_ps = [ps_o.tile([128, DV + 2], F32, tag="o", name="o_ps")
                for _ in range(4)]
        for kb in range(4):
            batches = (BATCHES_KB0 if kb == 0
                       else BATCHES_KB3 if kb == 3 else BATCHES_MID)
            sc_ps = ps_sc.tile([128, 512], F32, tag="sc")
            if kb > 0:
                # mask fold first (maskT ready by now); the last red MM
                # then closes the accumulation group, shortening the
                # exp's critical path at each block boundary.
                nc.tensor.matmul(
                    sc_ps[:], negib, maskT[kb][:],
                    start=True, stop=False, skip_group_check=True,
                )
            jj = 0  # pair index within this key block
            for bnum, nb in enumerate(batches):
                a_t = abatch.tile([128, nb * 512], BF16, tag="a", name="a_t",
                                  padded_shape=[128, 8192])
                for j in range(nb):
                    pair = kb * 64 + jj + j
                    nc.vector.tensor_scalar_add(
                        out=a_t[:, j * 512:(j + 1) * 512],
                        in0=xq[:],
                        scalar1=b2[:, pair:pair + 1],
                    )
                t_t = tbatch.tile([128, nb * 512], BF16, tag="t", name="t_t",
                                  padded_shape=[128, 8192])
                nc.scalar.activation(
                    t_t[:], a_t[:], mybir.ActivationFunctionType.Tanh
                )
                for j in range(nb):
                    last = (jj + j == 63) and kb > 0
                    nc.tensor.matmul(
                        sc_ps[:],
                        wvwin[:, 126 - 2 * (jj + j): 254 - 2 * (jj + j)],
                        t_t[:, j * 512:(j + 1) * 512],
                        start=(jj + j == 0) and kb == 0, stop=last,
                        skip_group_check=True,
                    )
                jj += nb
                if kb == 0 and 1 <= bnum <= 3:
                    # deferred prologue work rides between the early
                    # batches (its input DMAs land during the ramp)
                    emit_k_chain(bnum)
                elif kb == 0 and bnum == 4:
                    emit_mask_prep()
            if kb == 0:
                # fold the (q,k) mask: scoresT += (-BIG*I).T @ maskT[kb]
                nc.tensor.matmul(
                    sc_ps[:], negib, maskT[kb][:],
                    start=False, stop=True, skip_group_check=True,
                )
            e_t = epool.tile([128, 512], F32R, tag="e")
            nc.scalar.activation(
                e_t[:], sc_ps[:], mybir.ActivationFunctionType.Exp
            )
            for qb in range(4):
                nc.tensor.matmul(
                    o_ps[qb][:],
                    e_t[:, qb * 128:(qb + 1) * 128],
                    vo[kb],
                    start=(kb == 0), stop=(kb == 3),
                    skip_group_check=True,
                )

        # --- normalize and write out. Order: all recips (DVE), then the
        # scales (qb0/qb2 on ACT, qb1/qb3 on DVE), then the DMAs - so no
        # engine queue head-of-line-blocks on another's scale. ---
        recl, otl = [], []
        for qb in range(4):
            rec = recs.tile([128, 1], F32, tag="rec", name="rec", bufs=4)
            nc.vector.reciprocal(rec[:], o_ps[qb][:, DV:DV + 1])
            recl.append(rec)
        for qb in (0, 2, 1, 3):
            o_t = outp.tile([128, DV], F32, tag="out", name="o_t", bufs=4)
            if qb % 2 == 0:
                nc.scalar.activation(
                    o_t[:], o_ps[qb][:, 0:DV],
                    mybir.ActivationFunctionType.Copy, scale=recl[qb][:],
                )
            else:
                nc.vector.tensor_scalar_mul(
                    out=o_t[:], in0=o_ps[qb][:, 0:DV], scalar1=recl[qb][:]
                )
            otl.append((qb, o_t))
        for qb, o_t in sorted(otl):
            eng = nc.sync if qb % 2 == 0 else nc.scalar
            eng.dma_start(out=out_d[qb * 128:(qb + 1) * 128, :], in_=o_t[:])


def build():
    """Build + compile the (SPMD, per-core) Bass program. Cached."""
    if "nc" in _CACHE:
        return _CACHE["nc"]
    nc = bacc.Bacc("TRN2", target_bir_lowering=False, debug=False,
                   num_devices=NCORES)
    io = {
        "q": nc.dram_tensor("q", [LQ, D], F32, kind="ExternalInput"),
        "k": nc.dram_tensor("k", [LK, D], F32, kind="ExternalInput"),
        "vo": nc.dram_tensor("vo", [LK, DV + 2], F32, kind="ExternalInput"),
        "mask": nc.dram_tensor("mask", [LQ, LK], U8, kind="ExternalInput"),
        "constsf": nc.dram_tensor("constsf", [128, 512], F32,
                                  kind="ExternalInput"),
        "constsb": nc.dram_tensor("constsb", [128, 510], BF16,
                                  kind="ExternalInput"),
        "out": nc.dram_tensor("out", [LQ, DV], F32, kind="ExternalOutput"),
    }
    with tile.TileContext(nc) as tc:
        _emit(nc, tc, io)
    nc.compile()
    _CACHE["nc"] = nc
    return nc


def make_in_maps(queries, keys, values, mask, Wq, Wk, wv):
    queries = np.asarray(queries, dtype=np.float32)
    keys = np.asarray(keys, dtype=np.float32)
    values = np.asarray(values, dtype=np.float32)
    mask_u8 = np.ascontiguousarray(np.asarray(mask)).view(np.uint8)
    Wq = np.asarray(Wq, dtype=np.float32)
    Wk = np.asarray(Wk, dtype=np.float32)
    wv = np.asarray(wv, dtype=np.float32)

    constsf = np.zeros((128, 512), dtype=np.float32)
    constsf[:, 0:128] = np.concatenate([Wq[0:128], Wq[0:128]], axis=1)
    constsf[:, 128:256] = np.concatenate([Wq[128:256], Wq[128:256]], axis=1)
    constsf[:, 256:384] = np.concatenate([Wk[0:128], Wk[0:128]], axis=1)
    constsf[:, 384:512] = np.concatenate([Wk[128:256], Wk[128:256]], axis=1)

    constsb = np.zeros((128, 510), dtype=ml_dtypes.bfloat16)
    constsb[:, 0:128] = np.eye(128, dtype=ml_dtypes.bfloat16)
    constsb[:, 128:256] = (-BIGNEG * np.eye(128, dtype=np.float32)
                           ).astype(ml_dtypes.bfloat16)
    constsb[0:64, 256 + 126] = wv.astype(ml_dtypes.bfloat16)
    constsb[64:128, 256 + 127] = wv.astype(ml_dtypes.bfloat16)

    ones_col = np.ones((LK, 1), dtype=np.float32)
    in_maps = []
    for b in range(B):
        vo = np.ascontiguousarray(
            np.concatenate([values[b], ones_col,
                            np.zeros((LK, 1), np.float32)], axis=1),
            dtype=np.float32,
        )
        in_maps.append({
            "q": np.ascontiguousarray(queries[b]),
            "k": np.ascontiguousarray(keys[b]),
            "vo": vo,
            "mask": np.ascontiguousarray(mask_u8[b]),
            "constsf": constsf,
            "constsb": constsb,
        })
    return in_maps


def kernel(queries, keys, values, mask, Wq, Wk, wv, **run_kwargs):
    nc = build()
    in_maps = make_in_maps(queries, keys, values, mask, Wq, Wk, wv)
    res = run_bass_kernel_spmd(nc, in_maps, core_ids=list(range(NCORES)),
                               **run_kwargs)
    out = np.stack([r["out"] for r in res.results], axis=0)
    if run_kwargs:
        kernel.last_results = res
    return out.astype(np.float32)


# revision 37
# speedup vs baseline: 1.0049x; 1.0049x over previous
"""AdditiveAttention (Bahdanau) Trainium2 Bass kernel.

Math (per batch b):
  qf = queries @ Wq                  (Lq, H)
  kf = keys @ Wk                     (Lk, H)
  scores[q,k] = sum_h wv[h] * tanh(qf[q,h] + kf[k,h])
  attn = softmax(where(mask, -inf, scores), axis=k)
  out  = attn @ values               (Lq, Dv)

Sharding: data-parallel over batch B=8 across the 8 NeuronCores (one
batch per core). Everything is fused on-chip; the (B,Lq,Lk,H) feature
intermediate never touches HBM.

The dominant cost is the 512*512*64 = 16.8M tanh evaluations per core,
which only ScalarE (ACT) can do, at 1 elem/cycle/lane @ 1.2 GHz =
~109us/core. The whole kernel is organized to keep ACT saturated with
tanh from ~22us (DMA-bound ramp) to the end (~138us); measured
exec_time ~147us (the remainder: NEFF preamble, ramp, drain barrier).

Per-core dataflow (Lq=Lk=512, D=256, H=64; h2 = stacked 2x64 heads):
  - q/k land via one DMA per 128-row block spread over the 3 DMA rings
    (Sync/Scalar HWDGE + GpSimd SWDGE; per-DMA BW is ~55 GB/s, the
    rings run in parallel).
  - PE-transposes (f32r, via on-device identity) -> qT/kT (d-major);
    Xq = [Wq|Wq].T @ qT -> (128 h2, 512 q), evacuated to bf16.
  - Per key-block kbi: Kst[:,kbi] = [Wk|Wk].T @ kT[:,kbi]; strided
    evacuation into B2 (128 h2, 256 pairs) f32 where column p =
    [kf[2p,:]; kf[2p+1,:]]. Block 0 is emitted in the prologue (it
    gates the first tanh); blocks 1-3 + mask prep + values casts are
    emitted between the first key-block's tanh batches so their late
    DMAs can't head-of-line-block the DVE queue.
  - Main loop over 256 key-pairs in sub-batches (4,4,8,16,16... pairs):
      DVE:  A[:, j*512:(j+1)*512] = Xq + B2[:, p]       (bf16 out)
      ACT:  T = tanh(A)                                  (the roofline)
      PE:   scoresT_psum(128 k,512 q) += Wwin_p.T @ T_j  (bf16)
    Wwin_p is a 128-col window of a (128, 254) constant holding [wv;0]
    and [0;wv] at columns 126/127: the window places wv at output
    partitions 2p, 2p+1, i.e. a block-diagonal reduction over h that
    accumulates a full PSUM bank without partition-offset writes.
  - Mask: maskT (k,q) bf16 via PE transposes; folded into the score
    accumulation as an extra matmul  scoresT += (-BIG*I).T @ maskT
    (first in the group for kb>0, last for kb0).
  - ACT exp (PSUM->SBUF, f32r); PE: O[qb] += E[:, qb].T @ [values|1|0]
    (f32r) gives the unnormalized output and the softmax denominator
    in column 256 of one PSUM bank per q-block.
  - Epilogue: DVE reciprocals; per-partition scales split ACT/DVE;
    output DMAs split across the Sync/Scalar rings.

kernel(**inputs) takes the FULL unsharded inputs and returns the full
(8, 512, 256) float32 output. Host-side prep is limited to tiny
constant packing (wv window, -BIG*I, [W|W] duplication) and appending
the ones column to values.
"""

import numpy as np
import ml_dtypes

import concourse.bass as bass
import concourse.mybir as mybir
import concourse.tile as tile
from concourse import bacc
from concourse.bass_utils import run_bass_kernel_spmd
from concourse.masks import make_identity

B, LQ, LK = 8, 512, 512
D, H = 256, 64
DV = 256
NCORES = 8
BIGNEG = 1.0e30           # mask fill magnitude (exp(-BIGNEG) == 0.0 in f32)

F32 = mybir.dt.float32
F32R = mybir.dt.float32r
BF16 = mybir.dt.bfloat16
U8 = mybir.dt.uint8

# tanh sub-batch sizes (in key-pairs) for each of the 4 key blocks;
# 16-pair steady state amortizes the ~300-cycle ACT per-instruction
# overhead; the small first/last batches shorten pipeline ramp/drain.
BATCHES_KB0 = [4, 4, 8, 16, 16, 16]
BATCHES_MID = [16] * 4
BATCHES_KB3 = [16, 16, 16, 8, 4, 4]

_CACHE = {}


def _emit(nc, tc, io):
    from contextlib import ExitStack

    q_d, k_d, vo_d, mask_d = io["q"], io["k"], io["vo"], io["mask"]
    constsf_d, constsb_d = io["constsf"], io["constsb"]
    out_d = io["out"]

    with ExitStack() as ctx:
        ep = ctx.enter_context
        consts = ep(tc.tile_pool(name="consts", bufs=1))
        qkraw = ep(tc.tile_pool(name="qkraw", bufs=1))
        qkT = ep(tc.tile_pool(name="qkT", bufs=2))
        small = ep(tc.tile_pool(name="small", bufs=1))
        abatch = ep(tc.tile_pool(name="abatch", bufs=3))
        tbatch = ep(tc.tile_pool(name="tbatch", bufs=3))
        epool = ep(tc.tile_pool(name="epool", bufs=2))
        mwork = ep(tc.tile_pool(name="mwork", bufs=1))
        mtT = ep(tc.tile_pool(name="mtT", bufs=4))
        votiles = ep(tc.tile_pool(name="votiles", bufs=1))
        outp = ep(tc.tile_pool(name="outp", bufs=2))
        recs = ep(tc.tile_pool(name="recs", bufs=2))
        # PSUM: 2 score banks + 2 prologue/scratch banks + 4 output
        # accumulator banks = all 8 banks.
        ps_sc = ep(tc.tile_pool(name="ps_sc", bufs=2, space="PSUM"))
        ps_pre = ep(tc.tile_pool(name="ps_pre", bufs=2, space="PSUM"))
        ps_o = ep(tc.tile_pool(name="ps_o", bufs=4, space="PSUM"))

        # --- constants: identity built on-device; W / mask consts DMA'd ---
        # constsf: [Wq2_c0 | Wq2_c1 | Wk2_c0 | Wk2_c1]  (duplicated cols)
        # constsb: [identity_bf16 | -BIG*identity_bf16 | wv window (254)]
        identf = small.tile([128, 128], F32, tag="identf")
        make_identity(nc, identf[:])
        identr = small.tile([128, 128], F32R, tag="identr")
        nc.vector.tensor_copy(identr[:], identf[:])

        # --- queries/keys: one DMA per 128-row block. 3 parallel DMA
        # rings: GpSimd leads with the W constants + k block 0 (kb0's
        # bias chain), Sync carries q0/q2 + k1/k3, Scalar q1/q3 + k2. ---
        qre = q_d.rearrange("(b p) d -> p b d", b=4)
        kre = k_d.rearrange("(b p) d -> p b d", b=4)
        qraw = qkraw.tile([128, 4, 256], F32, tag="qraw")
        kraw = qkraw.tile([128, 4, 256], F32, tag="kraw")
        cf = consts.tile([128, 512], F32, tag="cf")
        cb = consts.tile([128, 510], BF16, tag="cb")
        nc.gpsimd.dma_start(out=kraw[:, 0, :], in_=kre[:, 0, :])
        nc.sync.dma_start(out=qraw[:, 0, :], in_=qre[:, 0, :])
        nc.scalar.dma_start(out=qraw[:, 1, :], in_=qre[:, 1, :])
        nc.sync.dma_start(out=qraw[:, 2, :], in_=qre[:, 2, :])
        nc.gpsimd.dma_start(out=qraw[:, 3, :], in_=qre[:, 3, :])
        nc.scalar.dma_start(out=cf[:], in_=constsf_d[:])
        nc.sync.dma_start(out=kraw[:, 1, :], in_=kre[:, 1, :])
        nc.scalar.dma_start(out=kraw[:, 2, :], in_=kre[:, 2, :])
        nc.sync.dma_start(out=kraw[:, 3, :], in_=kre[:, 3, :])
        nc.gpsimd.dma_start(out=cb[:], in_=constsb_d[:])
        identb = cb[:, 0:128]
        negib = cb[:, 128:256]
        wvwin = cb[:, 256:510]

        # f32r rounding copies (BIR requires f32r matmul inputs to come
        # from rounding producers)
        wr = small.tile([128, 512], F32R, tag="wr")
        nc.vector.tensor_copy(wr[:], cf[:])
        wq_r = [wr[:, 0:128], wr[:, 128:256]]
        wk_r = [wr[:, 256:384], wr[:, 384:512]]
        qraw_r = qkraw.tile([128, 4, 256], F32R, tag="qraw_r")
        kraw_r = qkraw.tile([128, 4, 256], F32R, tag="kraw_r")
        nc.vector.tensor_copy(kraw_r[:, 0, :], kraw[:, 0, :])
        for blk in range(4):
            nc.vector.tensor_copy(qraw_r[:, blk, :], qraw[:, blk, :])

        # --- transpose q on PE (f32r), fully per-q-block pipelined:
        # each block's transposes, ACT evacuations, Xq matmul pair and
        # xq copy complete as soon as that block's DMA lands (subtile
        # deps), so only the last block's chain sits on the ramp. ---
        qT = [qkT.tile([128, 512], F32R, tag="qkT", name="qT")
              for _ in range(2)]
        # xq_ps borrows a ps_o slot (freed before o_ps[3] is written)
        xq_ps = ps_o.tile([128, 512], F32, tag="o", name="xq_ps")
        bankq = [ps_pre.tile([128, 512], F32R, tag="pre", name="tq")
                 for _ in range(2)]
        xq = small.tile([128, 512], BF16, tag="xq")
        for blk in range(4):
            for db in range(2):
                nc.tensor.transpose(
                    bankq[db][:, blk * 128:(blk + 1) * 128],
                    qraw_r[:, blk, db * 128:(db + 1) * 128],
                    identr[:],
                )
                nc.scalar.copy(
                    qT[db][:, blk * 128:(blk + 1) * 128],
                    bankq[db][:, blk * 128:(blk + 1) * 128],
                )
                nc.tensor.matmul(
                    xq_ps[:, blk * 128:(blk + 1) * 128],
                    wq_r[db], qT[db][:, blk * 128:(blk + 1) * 128],
                    start=(db == 0), stop=(db == 1),
                )
            nc.scalar.copy(
                xq[:, blk * 128:(blk + 1) * 128],
                xq_ps[:, blk * 128:(blk + 1) * 128],
            )

        # --- k chains: transpose + Kst + B2 columns per key-block.
        # Only block 0 is emitted in the prologue (it gates the first
        # tanh); blocks 1-3 are deferred into the kb0 batch region so
        # their late DMAs cannot head-of-line-block the DVE queue. ---
        kst_ps = ps_sc.tile([128, 512], F32, tag="sc", name="kst_ps")
        b2 = small.tile([128, 256], F32, tag="b2")
        kTb = qkT.tile([128, 4, 256], F32R, tag="kTb")

        def emit_k_chain(kbi):
            if kbi > 0:
                nc.vector.tensor_copy(kraw_r[:, kbi, :], kraw[:, kbi, :])
            bank = ps_pre.tile([128, 256], F32R, tag="pre", name="tk")
            for db in range(2):
                nc.tensor.transpose(
                    bank[:, db * 128:(db + 1) * 128],
                    kraw_r[:, kbi, db * 128:(db + 1) * 128],
                    identr[:],
                )
            nc.vector.tensor_copy(kTb[:, kbi, :], bank[:])
            for db in range(2):
                nc.tensor.matmul(
                    kst_ps[:, kbi * 128:(kbi + 1) * 128],
                    wk_r[db], kTb[:, kbi, db * 128:(db + 1) * 128],
                    start=(db == 0), stop=(db == 1),
                )
            nc.vector.tensor_copy(
                b2[0:64, kbi * 64:(kbi + 1) * 64],
                kst_ps[0:64, kbi * 128:(kbi + 1) * 128:2])
            nc.vector.tensor_copy(
                b2[64:128, kbi * 64:(kbi + 1) * 64],
                kst_ps[64:128, kbi * 128 + 1:(kbi + 1) * 128:2])

        emit_k_chain(0)

        # --- values|ones and mask loads (GpSimd SWDGE queue) ---
        vot = votiles.tile([128, 4, DV + 2], F32, tag="vo")
        nc.gpsimd.dma_start(out=vot[:],
                            in_=vo_d.rearrange("(b p) d -> p b d", b=4))
        vot_r = votiles.tile([128, 4, DV + 2], F32R, tag="vor")
        vo = [vot_r[:, kb, :] for kb in range(4)]
        mu8 = mwork.tile([128, 4, 512], U8, tag="mu8")
        nc.gpsimd.dma_start(out=mu8[:],
                            in_=mask_d.rearrange("(b p) d -> p b d", b=4))
        mbf = mwork.tile([128, 4, 512], BF16, tag="mbf")
        maskT = [mtT.tile([128, 512], BF16, tag="mt", name="mt")
                 for _ in range(4)]

        def emit_mask_prep():
            # maskT (k, q) via banked PE transposes; emitted after kb0's
            # tanh batches so it does not steal PE/DVE from the ramp.
            nc.vector.tensor_copy(vot_r[:], vot[:])
            nc.vector.tensor_copy(mbf[:], mu8[:])
            for kb in range(4):
                bank = ps_pre.tile([128, 512], BF16, tag="pre", name="tm")
                for qb in range(4):
                    nc.tensor.transpose(
                        bank[:, qb * 128:(qb + 1) * 128],
                        mbf[:, qb, kb * 128:(kb + 1) * 128],
                        identb,
                    )
                nc.vector.tensor_copy(maskT[kb][:], bank[:])

        # --- main loop: tanh features + blockwise wv reduction ---
        o_ps = [ps_o.tile([128, DV + 2], F32, tag="o", name="o_ps")
                for _ in range(4)]
        for kb in range(4):
            batches = (BATCHES_KB0 if kb == 0
                       else BATCHES_KB3 if kb == 3 else BATCHES_MID)
            sc_ps = ps_sc.tile([128, 512], F32, tag="sc")
            if kb > 0:
                # mask fold first (maskT ready by now); the last red MM
                # then closes the accumulation group, shortening the
                # exp's critical path at each block boundary.
                nc.tensor.matmul(
                    sc_ps[:], negib, maskT[kb][:],
                    start=True, stop=False, skip_group_check=True,
                )
            jj = 0  # pair index within this key block
            for bnum, nb in enumerate(batches):
                a_t = abatch.tile([128, nb * 512], BF16, tag="a", name="a_t",
                                  padded_shape=[128, 8192])
                for j in range(nb):
                    pair = kb * 64 + jj + j
                    nc.vector.tensor_scalar_add(
                        out=a_t[:, j * 512:(j + 1) * 512],
                        in0=xq[:],
                        scalar1=b2[:, pair:pair + 1],
                    )
                t_t = tbatch.tile([128, nb * 512], BF16, tag="t", name="t_t",
                                  padded_shape=[128, 8192])
                nc.scalar.activation(
                    t_t[:], a_t[:], mybir.ActivationFunctionType.Tanh
                )
                for j in range(nb):
                    last = (jj + j == 63) and kb > 0
                    nc.tensor.matmul(
                        sc_ps[:],
                        wvwin[:, 126 - 2 * (jj + j): 254 - 2 * (jj + j)],
                        t_t[:, j * 512:(j + 1) * 512],
                        start=(jj + j == 0) and kb == 0, stop=last,
                        skip_group_check=True,
                    )
                jj += nb
                if kb == 0 and 1 <= bnum <= 3:
                    # deferred prologue work rides between the early
                    # batches (its input DMAs land during the ramp)
                    emit_k_chain(bnum)
                elif kb == 0 and bnum == 4:
                    emit_mask_prep()
            if kb == 0:
                # fold the (q,k) mask: scoresT += (-BIG*I).T @ maskT[kb]
                nc.tensor.matmul(
                    sc_ps[:], negib, maskT[kb][:],
                    start=False, stop=True, skip_group_check=True,
                )
            e_t = epool.tile([128, 512], F32R, tag="e")
            nc.scalar.activation(
                e_t[:], sc_ps[:], mybir.ActivationFunctionType.Exp
            )
            for qb in range(4):
                nc.tensor.matmul(
                    o_ps[qb][:],
                    e_t[:, qb * 128:(qb + 1) * 128],
                    vo[kb],
                    start=(kb == 0), stop=(kb == 3),
                    skip_group_check=True,
                )

        # --- normalize and write out. Order: all recips (DVE), then the
        # scales (qb0/qb2 on ACT, qb1/qb3 on DVE), then the DMAs - so no
        # engine queue head-of-line-blocks on another's scale. ---
        recl, otl = [], []
        for qb in range(4):
            rec = recs.tile([128, 1], F32, tag="rec", name="rec", bufs=4)
            nc.vector.reciprocal(rec[:], o_ps[qb][:, DV:DV + 1])
            recl.append(rec)
        for qb in (0, 2, 1, 3):
            o_t = outp.tile([128, DV], F32, tag="out", name="o_t", bufs=4)
            if qb % 2 == 0:
                nc.scalar.activation(
                    o_t[:], o_ps[qb][:, 0:DV],
                    mybir.ActivationFunctionType.Copy, scale=recl[qb][:],
                )
            else:
                nc.vector.tensor_scalar_mul(
                    out=o_t[:], in0=o_ps[qb][:, 0:DV], scalar1=recl[qb][:]
                )
            otl.append((qb, o_t))
        for qb, o_t in sorted(otl):
            eng = nc.sync if qb % 2 == 0 else nc.scalar
            eng.dma_start(out=out_d[qb * 128:(qb + 1) * 128, :], in_=o_t[:])


def build():
    """Build + compile the (SPMD, per-core) Bass program. Cached."""
    if "nc" in _CACHE:
        return _CACHE["nc"]
    nc = bacc.Bacc("TRN2", target_bir_lowering=False, debug=False,
                   num_devices=NCORES)
    io = {
        "q": nc.dram_tensor("q", [LQ, D], F32, kind="ExternalInput"),
        "k": nc.dram_tensor("k", [LK, D], F32, kind="ExternalInput"),
        "vo": nc.dram_tensor("vo", [LK, DV + 2], F32, kind="ExternalInput"),
        "mask": nc.dram_tensor("mask", [LQ, LK], U8, kind="ExternalInput"),
        "constsf": nc.dram_tensor("constsf", [128, 512], F32,
                                  kind="ExternalInput"),
        "constsb": nc.dram_tensor("constsb", [128, 510], BF16,
                                  kind="ExternalInput"),
        "out": nc.dram_tensor("out", [LQ, DV], F32, kind="ExternalOutput"),
    }
    with tile.TileContext(nc) as tc:
        _emit(nc, tc, io)
    nc.compile()
    _CACHE["nc"] = nc
    return nc


def make_in_maps(queries, keys, values, mask, Wq, Wk, wv):
    queries = np.asarray(queries, dtype=np.float32)
    keys = np.asarray(keys, dtype=np.float32)
    values = np.asarray(values, dtype=np.float32)
    mask_u8 = np.ascontiguousarray(np.asarray(mask)).view(np.uint8)
    Wq = np.asarray(Wq, dtype=np.float32)
    Wk = np.asarray(Wk, dtype=np.float32)
    wv = np.asarray(wv, dtype=np.float32)

    constsf = np.zeros((128, 512), dtype=np.float32)
    constsf[:, 0:128] = np.concatenate([Wq[0:128], Wq[0:128]], axis=1)
    constsf[:, 128:256] = np.concatenate([Wq[128:256], Wq[128:256]], axis=1)
    constsf[:, 256:384] = np.concatenate([Wk[0:128], Wk[0:128]], axis=1)
    constsf[:, 384:512] = np.concatenate([Wk[128:256], Wk[128:256]], axis=1)

    constsb = np.zeros((128, 510), dtype=ml_dtypes.bfloat16)
    constsb[:, 0:128] = np.eye(128, dtype=ml_dtypes.bfloat16)
    constsb[:, 128:256] = (-BIGNEG * np.eye(128, dtype=np.float32)
                           ).astype(ml_dtypes.bfloat16)
    constsb[0:64, 256 + 126] = wv.astype(ml_dtypes.bfloat16)
    constsb[64:128, 256 + 127] = wv.astype(ml_dtypes.bfloat16)

    ones_col = np.ones((LK, 1), dtype=np.float32)
    in_maps = []
    for b in range(B):
        vo = np.ascontiguousarray(
            np.concatenate([values[b], ones_col,
                            np.zeros((LK, 1), np.float32)], axis=1),
            dtype=np.float32,
        )
        in_maps.append({
            "q": np.ascontiguousarray(queries[b]),
            "k": np.ascontiguousarray(keys[b]),
            "vo": vo,
            "mask": np.ascontiguousarray(mask_u8[b]),
            "constsf": constsf,
            "constsb": constsb,
        })
    return in_maps


def kernel(queries, keys, values, mask, Wq, Wk, wv, **run_kwargs):
    nc = build()
    in_maps = make_in_maps(queries, keys, values, mask, Wq, Wk, wv)
    res = run_bass_kernel_spmd(nc, in_maps, core_ids=list(range(NCORES)),
                               **run_kwargs)
    out = np.stack([r["out"] for r in res.results], axis=0)
    if run_kwargs:
        kernel.last_results = res
    return out.astype(np.float32)
